# revision 4
# baseline (speedup 1.0000x reference)
"""GNN SAGEConv (mean-agg) Trainium2 kernel, 8-core SPMD.

Strategy (node-range sharding; no collectives):
  - Host: sort edges by dst, split into 8 node-aligned shards (~E/8 edges
    each), pad shards to a common edge count.
  - Device (per core): for each 128-edge chunk, gather x-rows of the chunk's
    sources via indirect DMA (one row per partition), compute the chunk-local
    prefix-sum over edges with one PE matmul against a constant triangular
    matrix, and write the prefix-sums to DRAM.  Per-node sums are then three
    indexed reads per node (run-end, run-start-1, chunk-straddle fixup):
    agg[n] = csum[end_n-1] - csum[start_n-1] + straddle-part.  A ones-column
    appended to x yields per-node counts in the same pass.  The epilogue
    computes out.T = (agg@W_l.T)*recip(cnt) + x@W_r.T + b via PE transposes
    and feature-major matmuls.
  - Host: concatenate the 8 per-core [12, nodes] outputs and transpose.
"""

from contextlib import ExitStack

import numpy as np

N_NODES = 100000
N_EDGES = 6400000
D = 12
DP = 16            # padded feature width (64B rows)
NCORES = 8
NPAD = 13056       # per-core node slots (102 * 128)
KN = NPAD // 128   # boundary-gather instructions per region
ZROW = N_NODES     # zero row of the x table
XROWS = 113280     # >= N_NODES + NPAD, multiple of 128
CHUNK = 128
C_PER_PHASE = 32   # chunks per phase (one PSUM bank: 32*16 = 512 f32)

_MAX_WAITS = 1


def _apply_tile_patches(tile_mod, mybir, vector_clock):
    """Walrus in this toolchain accepts at most one sync-wait per
    instruction; TileContext's exit drain aggregates one wait per DMA lane.
    Replace it with chained single-wait NOPs."""
    ScopedClock = vector_clock.ScopedClock

    def _drain_and_barrier(self, tick_clock, wait_clock):
        nc = self.nc
        probe = nc.sync.nop(hint="drain_wait_probe", nofuse=True)
        wait_clock.add_sem_waits(
            probe.ins, ScopedClock({None: tick_clock.global_clock})
        )
        si = probe.ins.sync_info
        waits = list(si.on_wait) if si is not None else []
        if len(waits) > _MAX_WAITS:
            si.on_wait = waits[:_MAX_WAITS]
            for i in range(_MAX_WAITS, len(waits), _MAX_WAITS):
                n = nc.sync.nop(hint="drain_wait_extra", nofuse=True)
                nsi = n.ins.sync_info
                if nsi is None:
                    n.ins.sync_info = mybir.SyncInfo(
                        on_wait=waits[i:i + _MAX_WAITS], on_update=[]
                    )
                else:
                    nsi.on_wait = waits[i:i + _MAX_WAITS]
        nc.sync.drain()
        nc.all_engine_barrier()
        assert self.sems is not None
        popped = nc._tile_sem_poison_stack.pop()
        assert popped is self._sem_poison
        nc.clear_and_free_semaphores(list(self.sems.allocated().values()))
        nc.all_engine_barrier()

    tile_mod.TileContext._drain_and_barrier = _drain_and_barrier


def _split_multi_waits(nc, mybir):
    cnt = 0
    for f in nc.m.functions:
        for bb in f.blocks:
            new = []
            for inst in bb.instructions:
                si = inst.sync_info
                waits = list(si.on_wait) if (si is not None and si.on_wait) else []
                if len(waits) > _MAX_WAITS:
                    extra, keep = waits[:-_MAX_WAITS], waits[-_MAX_WAITS:]
                    for j in range(0, len(extra), _MAX_WAITS):
                        nop = mybir.InstNoOp(name=f"waitsplit_{cnt}", ins=[], outs=[])
                        cnt += 1
                        nop.engine = inst.engine
                        nop.sync_info = mybir.SyncInfo(
                            on_wait=extra[j:j + _MAX_WAITS], on_update=[]
                        )
                        new.append(nop)
                    si.on_wait = keep
                new.append(inst)
            bb.instructions[:] = new


def _build_program(e_pad):
    import concourse.bass as bass
    import concourse.mybir as mybir
    import concourse.tile as tile
    import concourse.vector_clock as vector_clock

    _apply_tile_patches(tile, mybir, vector_clock)

    f32 = mybir.dt.float32
    i32 = mybir.dt.int32
    K = e_pad // CHUNK           # number of 128-edge chunks
    NPH = K // C_PER_PHASE       # phases

    nc = bass.Bass()
    xt = nc.declare_dram_parameter("xt", [XROWS, DP], f32, isOutput=False)
    xs = nc.declare_dram_parameter("xs", [NPAD, DP], f32, isOutput=False)
    gidx = nc.declare_dram_parameter("gidx", [128, K], i32, isOutput=False)
    bidx = nc.declare_dram_parameter("bidx", [128, 3 * KN], i32, isOutput=False)
    wl = nc.declare_dram_parameter("wl", [DP, D], f32, isOutput=False)
    wr = nc.declare_dram_parameter("wr", [DP, D], f32, isOutput=False)
    ltri = nc.declare_dram_parameter("ltri", [128, 128], f32, isOutput=False)
    ident = nc.declare_dram_parameter("ident", [128, 128], f32, isOutput=False)
    out = nc.declare_dram_parameter("out", [12, NPAD], f32, isOutput=True)
    # csum kept as an output so it lands in the DGE table (indirect-DMA base).
    csum = nc.declare_dram_parameter("csum", [e_pad + 128, DP], f32, isOutput=True)

    with ExitStack() as ctx:
        tc = ctx.enter_context(tile.TileContext(nc))
        const = ctx.enter_context(tc.tile_pool(name="const", bufs=1))
        msgs_p = ctx.enter_context(tc.tile_pool(name="msgs", bufs=3))
        cs_p = ctx.enter_context(tc.tile_pool(name="cs", bufs=3))
        ps_big = ctx.enter_context(tc.tile_pool(name="psb", bufs=3, space="PSUM"))
        ps_sm = ctx.enter_context(tc.tile_pool(name="pss", bufs=1, space="PSUM"))
        ep = ctx.enter_context(tc.tile_pool(name="ep", bufs=2))
        keep = ctx.enter_context(tc.tile_pool(name="keep", bufs=1))

        gidx_t = const.tile([128, K], i32)
        nc.gpsimd.dma_start(out=gidx_t[:], in_=gidx[:])
        bidx_t = const.tile([128, 3 * KN], i32)
        nc.gpsimd.dma_start(out=bidx_t[:], in_=bidx[:])
        wl_t = const.tile([DP, D], f32)
        nc.gpsimd.dma_start(out=wl_t[:], in_=wl[:])
        wr_t = const.tile([DP, D], f32)
        nc.gpsimd.dma_start(out=wr_t[:], in_=wr[:])
        lt_t = const.tile([128, 128], f32)
        nc.gpsimd.dma_start(out=lt_t[:], in_=ltri[:])
        id_t = const.tile([128, 128], f32)
        nc.gpsimd.dma_start(out=id_t[:], in_=ident[:])
        z_t = const.tile([128, DP], f32)
        nc.gpsimd.memset(z_t[:], 0.0)
        nc.sync.dma_start(out=csum[e_pad:e_pad + 128, :], in_=z_t[:])

        # ---- phase loop: gather 4096 msgs, chunk-local cumsum, store ----
        csv = csum[:e_pad, :].rearrange("(a c p) f -> a p c f", c=C_PER_PHASE, p=128)
        for ph in range(NPH):
            msgs = msgs_p.tile([128, C_PER_PHASE, DP], f32, tag="msgs")
            for c in range(C_PER_PHASE):
                k = ph * C_PER_PHASE + c
                nc.gpsimd.indirect_dma_start(
                    out=msgs[:, c, :],
                    out_offset=None,
                    in_=xt[:],
                    in_offset=bass.IndirectOffsetOnAxis(
                        ap=gidx_t[:, k:k + 1], axis=0
                    ),
                )
            mm = ps_big.tile([128, C_PER_PHASE * DP], f32, tag="mm")
            nc.tensor.matmul(
                mm[:], lt_t[:], msgs[:].rearrange("p c f -> p (c f)"),
                start=True, stop=True,
            )
            cs = cs_p.tile([128, C_PER_PHASE * DP], f32, tag="cs")
            if ph % 2 == 0:
                nc.vector.tensor_copy(cs[:], mm[:])
            else:
                nc.scalar.copy(cs[:], mm[:])
            nc.sync.dma_start(
                out=csv[ph],
                in_=cs[:].rearrange("p (c f) -> p c f", f=DP),
            )

        # ---- boundary gathers: L, S, P regions ----
        L_t = keep.tile([128, KN * DP], f32)
        S_t = keep.tile([128, KN * DP], f32)
        P_t = keep.tile([128, KN * DP], f32)
        for r, reg in enumerate((L_t, S_t, P_t)):
            for k in range(KN):
                nc.gpsimd.indirect_dma_start(
                    out=reg[:, k * DP:(k + 1) * DP],
                    out_offset=None,
                    in_=csum[:],
                    in_offset=bass.IndirectOffsetOnAxis(
                        ap=bidx_t[:, r * KN + k:r * KN + k + 1], axis=0
                    ),
                )
        agg = keep.tile([128, KN * DP], f32)
        nc.vector.tensor_add(out=agg[:], in0=L_t[:], in1=P_t[:])
        nc.vector.tensor_tensor(
            out=agg[:], in0=agg[:], in1=S_t[:], op=mybir.AluOpType.subtract
        )

        # ---- epilogue: out.T = (mean@Wl.T + x@Wr.T + b).T, mean = agg*recip ----
        rec = keep.tile([128, KN], f32)
        aggv = agg[:].rearrange("p (k f) -> p k f", f=DP)
        nc.vector.tensor_scalar_max(rec[:], aggv[:, :, D], 1.0)
        nc.vector.reciprocal(rec[:], rec[:])

        outT = keep.tile([12, NPAD], f32)
        xsv = xs[:].rearrange("(k p) f -> p k f", p=128)
        groups = [(g * 4, min(4, KN - g * 4)) for g in range((KN + 3) // 4)]
        for g0, gw in groups:
            n_w = gw * 128
            xp = ep.tile([128, 4 * DP], f32, tag="xp")
            nc.sync.dma_start(
                out=xp[:, :gw * DP].rearrange("p (k f) -> p k f", f=DP),
                in_=xsv[:, g0:g0 + gw, :],
            )
            mean = ep.tile([128, 4 * DP], f32, tag="mean")
            for t in range(gw):
                nc.vector.tensor_scalar_mul(
                    mean[:, t * DP:(t + 1) * DP],
                    agg[:, (g0 + t) * DP:(g0 + t + 1) * DP],
                    rec[:, g0 + t:g0 + t + 1],
                )
            aT_ps = ps_sm.tile([DP, 512], f32, tag="aT")
            xT_ps = ps_sm.tile([DP, 512], f32, tag="xT")
            for t in range(gw):
                nc.tensor.transpose(
                    out=aT_ps[:, t * 128:(t + 1) * 128],
                    in_=mean[:, t * DP:(t + 1) * DP],
                    identity=id_t[:],
                )
                nc.tensor.transpose(
                    out=xT_ps[:, t * 128:(t + 1) * 128],
                    in_=xp[:, t * DP:(t + 1) * DP],
                    identity=id_t[:],
                )
            aT = ep.tile([DP, 512], f32, tag="aTs")
            xT = ep.tile([DP, 512], f32, tag="xTs")
            nc.vector.tensor_copy(aT[:, :n_w], aT_ps[:, :n_w])
            nc.scalar.copy(xT[:, :n_w], xT_ps[:, :n_w])
            o1 = ps_sm.tile([12, 512], f32, tag="o1")
            nc.tensor.matmul(o1[:, :n_w], wl_t[:], aT[:, :n_w],
                             start=True, stop=False)
            nc.tensor.matmul(o1[:, :n_w], wr_t[:], xT[:, :n_w],
                             start=False, stop=True)
            nc.vector.tensor_copy(outT[:, g0 * 128:g0 * 128 + n_w], o1[:, :n_w])
        nc.sync.dma_start(out=out[:], in_=outT[:])

    _split_multi_waits(nc, mybir)
    return nc


def kernel(x, W_l, W_r, b, edge_index):
    from concourse.bass_utils import run_bass_kernel_spmd

    x = np.asarray(x, dtype=np.float32)
    W_l = np.asarray(W_l, dtype=np.float32)
    W_r = np.asarray(W_r, dtype=np.float32)
    b = np.asarray(b, dtype=np.float32)
    src = np.asarray(edge_index[0], dtype=np.int64)
    dst = np.asarray(edge_index[1], dtype=np.int64)
    E = src.shape[0]

    # ---- host prep: sort by dst, shard by node range ----
    order = np.argsort(dst, kind="stable")
    src_s = src[order].astype(np.int32)
    dst_s = dst[order].astype(np.int32)

    pos = [0]
    for i in range(1, NCORES):
        t = (i * E) // NCORES
        v = dst_s[min(t, E - 1)]
        pos.append(int(np.searchsorted(dst_s, v, side="left")))
    pos.append(E)
    nb = [int(dst_s[pos[i]]) if pos[i] < E else N_NODES for i in range(NCORES)]
    nb.append(N_NODES)

    e_cnt = [pos[i + 1] - pos[i] for i in range(NCORES)]
    e_pad = -(-max(e_cnt) // (CHUNK * C_PER_PHASE)) * (CHUNK * C_PER_PHASE)
    K = e_pad // CHUNK
    ZCS = e_pad  # zero row of csum

    xt_np = np.zeros((XROWS, DP), dtype=np.float32)
    xt_np[:N_NODES, :D] = x
    xt_np[:N_NODES, D] = 1.0

    wl_np = np.zeros((DP, D), dtype=np.float32)
    wl_np[:D, :] = W_l.T
    wr_np = np.zeros((DP, D), dtype=np.float32)
    wr_np[:D, :] = W_r.T
    wr_np[D, :] = b  # rides the ones column of x
    lt_np = np.triu(np.ones((128, 128), dtype=np.float32))  # [e_in<=e_out]
    id_np = np.eye(128, dtype=np.float32)

    in_maps = []
    for i in range(NCORES):
        n0, n1 = nb[i], nb[i + 1]
        nn = n1 - n0
        assert nn <= NPAD, (nn, NPAD)
        seg = slice(pos[i], pos[i + 1])
        ec = e_cnt[i]

        src_pad = np.full(e_pad, ZROW, dtype=np.int32)
        src_pad[:ec] = src_s[seg]
        gidx_np = np.ascontiguousarray(src_pad.reshape(K, 128).T)

        dst_l = (dst_s[seg] - n0).astype(np.int64)
        cnt = np.bincount(dst_l, minlength=NPAD)
        ends = np.cumsum(cnt)
        starts = ends - cnt
        nz = cnt > 0
        c1 = starts // CHUNK
        c2 = np.maximum(ends - 1, 0) // CHUNK
        assert np.all((c2 - c1)[nz] <= 1), "node run spans >2 chunks"
        Lx = np.where(nz, ends - 1, ZCS)
        Sx = np.where(nz & (starts % CHUNK != 0), starts - 1, ZCS)
        Px = np.where(nz & (c2 > c1), c1 * CHUNK + (CHUNK - 1), ZCS)
        bidx_np = np.ascontiguousarray(
            np.concatenate(
                [a.astype(np.int32).reshape(KN, 128).T for a in (Lx, Sx, Px)],
                axis=1,
            )
        )

        xs_np = np.zeros((NPAD, DP), dtype=np.float32)
        xs_np[:min(NPAD, XROWS - n0)] = xt_np[n0:n0 + NPAD]

        in_maps.append({
            "xt": xt_np, "xs": xs_np, "gidx": gidx_np, "bidx": bidx_np,
            "wl": wl_np, "wr": wr_np, "ltri": lt_np, "ident": id_np,
        })

    nc = _build_program(e_pad)
    res = run_bass_kernel_spmd(
        nc, in_maps, core_ids=list(range(NCORES)), trace=True
    )
    if res.exec_time_ns:
        print(f"HW exec time: {res.exec_time_ns} ns")

    out = np.empty((N_NODES, D), dtype=np.float32)
    for i in range(NCORES):
        n0, n1 = nb[i], nb[i + 1]
        out[n0:n1, :] = res.results[i]["out"][:, :n1 - n0].T
    return out
